# revision 12
# baseline (speedup 1.0000x reference)
"""Trainium2 Bass kernel for ConcatHandshaking.

out[b, p, :] = tanh(hidden[b, i_p] @ W1.T + hidden[b, j_p] @ W2.T + fc_b)
for the S*(S+1)/2 upper-triangular pairs (i_p, j_p), i-major order.

Device layout: output features (H=768) on SBUF partitions, pair index on the
free dim.  Then the pair-add is `q2T[:, j] + p1T[:, i]` where the second term
is a per-partition scalar -> one DVE tensor_scalar_add per triu segment,
fused bias, one big ACT tanh per output chunk, large contiguous DMA writes.

Sharding (8 cores): core k handles batch b = k//2 and output-feature rows
[384*(k%2), 384*(k%2)+384) -> 3 stripes of [128 features, 32896 pairs] each.
Per-core DRAM output is (3, 128, 32896); host reassembles + transposes.

All per-core inputs are packed into ONE dram tensor (one DMA): the S3 LW
struct only carries a single sync wait, so the first matmul may depend on at
most one semaphore.
"""

import sys

import numpy as np

for _p in ("/opt/trn_rl_repo",):
    if _p not in sys.path:
        sys.path.insert(0, _p)

B, S, H = 4, 256, 768
P = S * (S + 1) // 2  # 32896
NCHUNK = 4
CHUNK = P // NCHUNK  # 8224
KT = H // 128  # 6 k-tiles
OC = 3  # o-chunks (of 128) per core
# packed input columns: [ ht (S) | w1t (384) | w2t (384) | fcb (1) | zeros (1) ]
W1C = S
W2C = S + 128 * OC
FBC = S + 2 * 128 * OC
ZC = FBC + 1
IC = ZC + 1  # 1026 columns

_NC_CACHE = {}
LAST = {}


def _chunk_pieces():
    """For each output chunk, (i, src0, src1, dst0):
    out_chunk[:, dst0:dst0+(src1-src0)] = q2T[:, src0:src1] + p1T[:, i]."""
    pieces = [[] for _ in range(NCHUNK)]
    off = 0
    for i in range(S):
        seg0, seg1 = off, off + (S - i)
        off = seg1
        c = seg0 // CHUNK
        while c * CHUNK < seg1:
            s = max(seg0, c * CHUNK)
            e = min(seg1, (c + 1) * CHUNK)
            src0 = i + (s - seg0)  # free index in q2T is j itself
            pieces[c].append((i, src0, src0 + (e - s), s - c * CHUNK))
            c += 1
    return pieces


def _build_nc(loop_k=None):
    import contextlib

    import concourse.bacc as bacc
    import concourse.mybir as mybir
    import concourse.tile as tile

    f32 = mybir.dt.float32
    # Bacc (not raw Bass): its compile() runs generate_event_semaphores,
    # which splits multi-sem waits to satisfy TRN2's 1-wait-per-instruction.
    nc = bacc.Bacc()

    inp_d = nc.declare_dram_parameter("inp", [H, IC], f32, isOutput=False)
    out_d = nc.declare_dram_parameter("out", [OC, 128, P], f32, isOutput=True)

    pieces = _chunk_pieces()
    Tanh = mybir.ActivationFunctionType.Tanh

    with tile.TileContext(nc) as tc:
        with (
            tc.tile_pool(name="const", bufs=1) as cpool,
            tc.tile_pool(name="mm", bufs=2, space="PSUM") as mpool,
            tc.tile_pool(name="outp", bufs=2) as opool,
            tc.For_i(0, loop_k, 1) if loop_k else contextlib.nullcontext(),
        ):
            inp_b = cpool.tile([128, KT * IC], f32, name="inp_b")
            nc.sync.dma_start(
                inp_b[:].rearrange("p (t c) -> p t c", t=KT),
                inp_d.rearrange("(t p) c -> p t c", p=128),
            )
            # block kk occupies cols [kk*IC, (kk+1)*IC)
            ht_t = [inp_b[:, kk * IC : kk * IC + S] for kk in range(KT)]
            fcb_t = [inp_b[:, c * IC + FBC : c * IC + FBC + 1] for c in range(OC)]

            # DVE absorbs the input-DMA semaphore with a single-wait copy;
            # zbias (all zeros) is then the bias source for every ACT op so
            # the scalar engine only ever waits on the DVE semaphore.
            zbias = cpool.tile([128, 1], f32, name="zbias")
            nc.vector.tensor_copy(zbias[:], inp_b[:, ZC : ZC + 1])

            p1_t, q2_t = [], []
            for c in range(OC):
                pm1 = mpool.tile([128, S], f32, name="pm1")
                pm2 = mpool.tile([128, S], f32, name="pm2")
                for kk in range(KT):
                    nc.tensor.matmul(
                        pm1[:],
                        inp_b[:, kk * IC + W1C + c * 128 : kk * IC + W1C + (c + 1) * 128],
                        ht_t[kk],
                        start=(kk == 0),
                        stop=(kk == KT - 1),
                    )
                for kk in range(KT):
                    nc.tensor.matmul(
                        pm2[:],
                        inp_b[:, kk * IC + W2C + c * 128 : kk * IC + W2C + (c + 1) * 128],
                        ht_t[kk],
                        start=(kk == 0),
                        stop=(kk == KT - 1),
                    )
                p1 = cpool.tile([128, S], f32, name=f"p1_{c}")
                q2 = cpool.tile([128, S], f32, name=f"q2_{c}")
                nc.vector.tensor_copy(p1[:], pm1[:])
                nc.vector.tensor_scalar_add(q2[:], pm2[:], fcb_t[c])
                p1_t.append(p1)
                q2_t.append(q2)

            for c in range(OC):
                for ci in range(NCHUNK):
                    ot = opool.tile([128, CHUNK], f32, name="ot")
                    for (i, s0, s1, d0) in pieces[ci]:
                        nc.vector.tensor_scalar_add(
                            ot[:, d0 : d0 + (s1 - s0)],
                            q2_t[c][:, s0:s1],
                            p1_t[c][:, i : i + 1],
                        )
                    # separate ACT output tile: the out-DMA must depend on
                    # exactly one semaphore (ACT), not DVE+ACT via in-place
                    ot2 = opool.tile([128, CHUNK], f32, name="ot2")
                    nc.scalar.activation(ot2[:], ot[:], Tanh, bias=zbias[:])
                    nc.sync.dma_start(
                        out_d[c, :, ci * CHUNK : (ci + 1) * CHUNK], ot2[:]
                    )
    nc.compile()
    return nc


def _get_nc():
    if "nc" not in _NC_CACHE:
        _NC_CACHE["nc"] = _build_nc()
    return _NC_CACHE["nc"]


def kernel(hidden_state, fc_w, fc_b, _trace=False, **_trace_kwargs):
    from concourse.bass_utils import run_bass_kernel_spmd

    hidden_state = np.asarray(hidden_state, dtype=np.float32)
    fc_w = np.asarray(fc_w, dtype=np.float32)
    fc_b = np.asarray(fc_b, dtype=np.float32)

    in_maps = []
    for k in range(8):
        b, h0 = k // 2, 384 * (k % 2)
        inp = np.zeros((H, IC), dtype=np.float32)
        inp[:, :S] = hidden_state[b].T
        inp[:, W1C : W1C + 384] = fc_w[h0 : h0 + 384, :H].T
        inp[:, W2C : W2C + 384] = fc_w[h0 : h0 + 384, H:].T
        inp[: 128 * OC, FBC] = fc_b[h0 : h0 + 384]
        in_maps.append(dict(inp=inp))

    nc = _get_nc()
    res = run_bass_kernel_spmd(
        nc, in_maps, core_ids=list(range(8)), trace=_trace, **_trace_kwargs
    )
    LAST["res"] = res

    full = np.empty((B, H, P), dtype=np.float32)
    for k in range(8):
        b, h0 = k // 2, 384 * (k % 2)
        full[b, h0 : h0 + 384] = res.results[k]["out"].reshape(384, P)
    return np.ascontiguousarray(full.transpose(0, 2, 1))


# revision 13
# speedup vs baseline: 1.2884x; 1.2884x over previous
"""Trainium2 Bass kernel for ConcatHandshaking.

out[b, p, :] = tanh(hidden[b, i_p] @ W1.T + hidden[b, j_p] @ W2.T + fc_b)
for the S*(S+1)/2 upper-triangular pairs (i_p, j_p), i-major order.

Device layout: output features (H=768) on SBUF partitions, pair index on the
free dim.  Then the pair-add is `q2T[:, j] + p1T[:, i]` where the second term
is a per-partition scalar -> one DVE tensor_scalar_add per triu segment,
fused bias, one big ACT tanh per output chunk, large contiguous DMA writes.

Sharding (8 cores): core k handles batch b = k//2 and output-feature rows
[384*(k%2), 384*(k%2)+384) -> 3 stripes of [128 features, 32896 pairs] each.
Per-core DRAM output is (3, 128, 32896); host reassembles + transposes.

Matmul operands ship as one bf16 tensor (PE 4x faster than f32; rel err
~1e-3 after f32 PSUM accumulation); fcb/zeros ship in a tiny f32 tensor.
The first stripe uses small leading chunks so the first output DMA starts
~12us in instead of waiting on a full 8224-wide chunk.
"""

import sys

import numpy as np

for _p in ("/opt/trn_rl_repo",):
    if _p not in sys.path:
        sys.path.insert(0, _p)

B, S, H = 4, 256, 768
P = S * (S + 1) // 2  # 32896
KT = H // 128  # 6 k-tiles
OC = 3  # o-chunks (of 128) per core
# bf16 packed matmul input columns: [ ht (S) | w1t (384) | w2t (384) ]
W1C = S
W2C = S + 128 * OC
IC16 = S + 2 * 128 * OC  # 1024
BIGCHUNK = 8224
SMALL = 2056

_NC_CACHE = {}
LAST = {}


def _stripe_chunks(c):
    if c == 0:
        return [SMALL] * 4 + [BIGCHUNK] * 3
    return [BIGCHUNK] * 4


def _chunk_pieces(chunk_list):
    """Split triu segments along chunk boundaries.

    Returns per-chunk lists of (i, src0, src1, dst0):
    chunk[:, dst0:dst0+(src1-src0)] = q2T[:, src0:src1] + p1T[:, i].
    """
    bounds = [0]
    for sz in chunk_list:
        bounds.append(bounds[-1] + sz)
    assert bounds[-1] == P
    pieces = [[] for _ in chunk_list]
    off = 0
    for i in range(S):
        seg0, seg1 = off, off + (S - i)
        off = seg1
        for ci, (c0, c1) in enumerate(zip(bounds[:-1], bounds[1:])):
            s = max(seg0, c0)
            e = min(seg1, c1)
            if e > s:
                src0 = i + (s - seg0)  # free index in q2T is j itself
                pieces[ci].append((i, src0, src0 + (e - s), s - c0))
    return pieces


def _build_nc(loop_k=None):
    import contextlib

    import concourse.bacc as bacc
    import concourse.mybir as mybir
    import concourse.tile as tile

    f32 = mybir.dt.float32
    bf16 = mybir.dt.bfloat16
    # Bacc (not raw Bass): its compile() runs generate_event_semaphores,
    # which splits multi-sem waits to satisfy TRN2's 1-wait-per-instruction.
    nc = bacc.Bacc()

    inp16_d = nc.declare_dram_parameter("inp16", [H, IC16], bf16, isOutput=False)
    # f32 side data: col 0 = fcb (rows 0:384), col 1 = zeros
    aux_d = nc.declare_dram_parameter("aux", [H, 2], f32, isOutput=False)
    out_d = nc.declare_dram_parameter("out", [OC, 128, P], f32, isOutput=True)

    Tanh = mybir.ActivationFunctionType.Tanh

    with tile.TileContext(nc) as tc:
        with (
            tc.tile_pool(name="const", bufs=1) as cpool,
            tc.tile_pool(name="mm", bufs=2, space="PSUM") as mpool,
            tc.tile_pool(name="outp", bufs=2) as opool,
            tc.For_i(0, loop_k, 1) if loop_k else contextlib.nullcontext(),
        ):
            inp_b = cpool.tile([128, KT * IC16], bf16, name="inp_b")
            nc.sync.dma_start(
                inp_b[:].rearrange("p (t c) -> p t c", t=KT),
                inp16_d.rearrange("(t p) c -> p t c", p=128),
            )
            aux_b = cpool.tile([128, KT * 2], f32, name="aux_b")
            nc.sync.dma_start(
                aux_b[:].rearrange("p (t c) -> p t c", t=KT),
                aux_d.rearrange("(t p) c -> p t c", p=128),
            )
            # block kk occupies cols [kk*IC16, (kk+1)*IC16)
            ht_t = [inp_b[:, kk * IC16 : kk * IC16 + S] for kk in range(KT)]
            fcb_t = [aux_b[:, c * 2 : c * 2 + 1] for c in range(OC)]
            zbias = aux_b[:, 1:2]

            p1_t, q2_t = [], []
            for c in range(OC):
                pm1 = mpool.tile([128, S], f32, name="pm1")
                pm2 = mpool.tile([128, S], f32, name="pm2")
                for kk in range(KT):
                    nc.tensor.matmul(
                        pm1[:],
                        inp_b[
                            :, kk * IC16 + W1C + c * 128 : kk * IC16 + W1C + (c + 1) * 128
                        ],
                        ht_t[kk],
                        start=(kk == 0),
                        stop=(kk == KT - 1),
                    )
                for kk in range(KT):
                    nc.tensor.matmul(
                        pm2[:],
                        inp_b[
                            :, kk * IC16 + W2C + c * 128 : kk * IC16 + W2C + (c + 1) * 128
                        ],
                        ht_t[kk],
                        start=(kk == 0),
                        stop=(kk == KT - 1),
                    )
                p1 = cpool.tile([128, S], f32, name=f"p1_{c}")
                q2 = cpool.tile([128, S], f32, name=f"q2_{c}")
                nc.vector.tensor_copy(p1[:], pm1[:])
                nc.vector.tensor_scalar_add(q2[:], pm2[:], fcb_t[c])
                p1_t.append(p1)
                q2_t.append(q2)

            for c in range(OC):
                chunk_list = _stripe_chunks(c)
                pieces = _chunk_pieces(chunk_list)
                coff = 0
                for ci, csz in enumerate(chunk_list):
                    ot = opool.tile([128, BIGCHUNK], f32, name="ot")
                    for (i, s0, s1, d0) in pieces[ci]:
                        nc.vector.tensor_scalar_add(
                            ot[:, d0 : d0 + (s1 - s0)],
                            q2_t[c][:, s0:s1],
                            p1_t[c][:, i : i + 1],
                        )
                    # separate ACT output tile: the out-DMA must depend on
                    # exactly one semaphore (ACT), not DVE+ACT via in-place
                    ot2 = opool.tile([128, BIGCHUNK], f32, name="ot2")
                    nc.scalar.activation(ot2[:, :csz], ot[:, :csz], Tanh, bias=zbias)
                    nc.sync.dma_start(
                        out_d[c, :, coff : coff + csz], ot2[:, :csz]
                    )
                    coff += csz
    nc.compile()
    return nc


def _get_nc():
    if "nc" not in _NC_CACHE:
        _NC_CACHE["nc"] = _build_nc()
    return _NC_CACHE["nc"]


def _make_in_maps(hidden_state, fc_w, fc_b):
    import ml_dtypes

    in_maps = []
    for k in range(8):
        b, h0 = k // 2, 384 * (k % 2)
        inp16 = np.empty((H, IC16), dtype=ml_dtypes.bfloat16)
        inp16[:, :S] = hidden_state[b].T.astype(ml_dtypes.bfloat16)
        inp16[:, W1C : W1C + 384] = fc_w[h0 : h0 + 384, :H].T.astype(
            ml_dtypes.bfloat16
        )
        inp16[:, W2C : W2C + 384] = fc_w[h0 : h0 + 384, H:].T.astype(
            ml_dtypes.bfloat16
        )
        aux = np.zeros((H, 2), dtype=np.float32)
        aux[: 128 * OC, 0] = fc_b[h0 : h0 + 384]
        in_maps.append(dict(inp16=inp16, aux=aux))
    return in_maps


def kernel(hidden_state, fc_w, fc_b, _trace=False, **_trace_kwargs):
    from concourse.bass_utils import run_bass_kernel_spmd

    hidden_state = np.asarray(hidden_state, dtype=np.float32)
    fc_w = np.asarray(fc_w, dtype=np.float32)
    fc_b = np.asarray(fc_b, dtype=np.float32)

    in_maps = _make_in_maps(hidden_state, fc_w, fc_b)
    nc = _get_nc()
    res = run_bass_kernel_spmd(
        nc, in_maps, core_ids=list(range(8)), trace=_trace, **_trace_kwargs
    )
    LAST["res"] = res

    full = np.empty((B, H, P), dtype=np.float32)
    for k in range(8):
        b, h0 = k // 2, 384 * (k % 2)
        full[b, h0 : h0 + 384] = res.results[k]["out"].reshape(384, P)
    return np.ascontiguousarray(full.transpose(0, 2, 1))


# revision 14
# speedup vs baseline: 1.2904x; 1.0016x over previous
"""Trainium2 Bass kernel for ConcatHandshaking.

out[b, p, :] = tanh(hidden[b, i_p] @ W1.T + hidden[b, j_p] @ W2.T + fc_b)
for the S*(S+1)/2 upper-triangular pairs (i_p, j_p), i-major order.

Device layout: output features (H=768) on SBUF partitions, pair index on the
free dim.  Then the pair-add is `q2T[:, j] + p1T[:, i]` where the second term
is a per-partition scalar -> one DVE tensor_scalar_add per triu segment,
fused bias, one big ACT tanh per output chunk, large contiguous DMA writes.

Sharding (8 cores): core k handles batch b = k//2 and output-feature rows
[384*(k%2), 384*(k%2)+384) -> 3 stripes of [128 features, 32896 pairs] each.
Per-core DRAM output is (3, 128, 32896); host reassembles + transposes.

Matmul operands ship as one bf16 tensor (PE 4x faster than f32; rel err
~1e-3 after f32 PSUM accumulation); fcb/zeros ship in a tiny f32 tensor.
The first stripe uses small leading chunks so the first output DMA starts
~12us in instead of waiting on a full 8224-wide chunk.
"""

import sys

import numpy as np

for _p in ("/opt/trn_rl_repo",):
    if _p not in sys.path:
        sys.path.insert(0, _p)

B, S, H = 4, 256, 768
P = S * (S + 1) // 2  # 32896
KT = H // 128  # 6 k-tiles
OC = 3  # o-chunks (of 128) per core
# bf16 packed matmul input columns: [ ht (S) | w1t (384) | w2t (384) ]
W1C = S
W2C = S + 128 * OC
IC16 = S + 2 * 128 * OC  # 1024
BIGCHUNK = 8224
SMALL = 2056

_NC_CACHE = {}
LAST = {}


def _stripe_chunks(c):
    if c == 0:
        # small leading chunks: first output DMA launches early
        return [1028, 1028, 2056, 2056, 2056, 8224, 8224, 8224]
    if c == OC - 1:
        # descending trailing chunks: shrink the exposed final-DMA tail
        return [8224, 8224, 8224, 4112, 2056, 1028, 1028]
    return [BIGCHUNK] * 4


def _chunk_pieces(chunk_list):
    """Split triu segments along chunk boundaries.

    Returns per-chunk lists of (i, src0, src1, dst0):
    chunk[:, dst0:dst0+(src1-src0)] = q2T[:, src0:src1] + p1T[:, i].
    """
    bounds = [0]
    for sz in chunk_list:
        bounds.append(bounds[-1] + sz)
    assert bounds[-1] == P
    pieces = [[] for _ in chunk_list]
    off = 0
    for i in range(S):
        seg0, seg1 = off, off + (S - i)
        off = seg1
        for ci, (c0, c1) in enumerate(zip(bounds[:-1], bounds[1:])):
            s = max(seg0, c0)
            e = min(seg1, c1)
            if e > s:
                src0 = i + (s - seg0)  # free index in q2T is j itself
                pieces[ci].append((i, src0, src0 + (e - s), s - c0))
    return pieces


def _build_nc(loop_k=None):
    import contextlib

    import concourse.bacc as bacc
    import concourse.mybir as mybir
    import concourse.tile as tile

    f32 = mybir.dt.float32
    bf16 = mybir.dt.bfloat16
    # Bacc (not raw Bass): its compile() runs generate_event_semaphores,
    # which splits multi-sem waits to satisfy TRN2's 1-wait-per-instruction.
    nc = bacc.Bacc()

    inp16_d = nc.declare_dram_parameter("inp16", [H, IC16], bf16, isOutput=False)
    # f32 side data: col 0 = fcb (rows 0:384), col 1 = zeros
    aux_d = nc.declare_dram_parameter("aux", [H, 2], f32, isOutput=False)
    out_d = nc.declare_dram_parameter("out", [OC, 128, P], f32, isOutput=True)

    Tanh = mybir.ActivationFunctionType.Tanh

    with tile.TileContext(nc) as tc:
        with (
            tc.tile_pool(name="const", bufs=1) as cpool,
            tc.tile_pool(name="mm", bufs=2, space="PSUM") as mpool,
            tc.tile_pool(name="outp", bufs=2) as opool,
            tc.For_i(0, loop_k, 1) if loop_k else contextlib.nullcontext(),
        ):
            inp_b = cpool.tile([128, KT * IC16], bf16, name="inp_b")
            nc.sync.dma_start(
                inp_b[:].rearrange("p (t c) -> p t c", t=KT),
                inp16_d.rearrange("(t p) c -> p t c", p=128),
            )
            aux_b = cpool.tile([128, KT * 2], f32, name="aux_b")
            nc.sync.dma_start(
                aux_b[:].rearrange("p (t c) -> p t c", t=KT),
                aux_d.rearrange("(t p) c -> p t c", p=128),
            )
            # block kk occupies cols [kk*IC16, (kk+1)*IC16)
            ht_t = [inp_b[:, kk * IC16 : kk * IC16 + S] for kk in range(KT)]
            fcb_t = [aux_b[:, c * 2 : c * 2 + 1] for c in range(OC)]
            zbias = aux_b[:, 1:2]

            p1_t, q2_t = [], []
            for c in range(OC):
                pm1 = mpool.tile([128, S], f32, name="pm1")
                pm2 = mpool.tile([128, S], f32, name="pm2")
                for kk in range(KT):
                    nc.tensor.matmul(
                        pm1[:],
                        inp_b[
                            :, kk * IC16 + W1C + c * 128 : kk * IC16 + W1C + (c + 1) * 128
                        ],
                        ht_t[kk],
                        start=(kk == 0),
                        stop=(kk == KT - 1),
                    )
                for kk in range(KT):
                    nc.tensor.matmul(
                        pm2[:],
                        inp_b[
                            :, kk * IC16 + W2C + c * 128 : kk * IC16 + W2C + (c + 1) * 128
                        ],
                        ht_t[kk],
                        start=(kk == 0),
                        stop=(kk == KT - 1),
                    )
                p1 = cpool.tile([128, S], f32, name=f"p1_{c}")
                q2 = cpool.tile([128, S], f32, name=f"q2_{c}")
                nc.vector.tensor_copy(p1[:], pm1[:])
                nc.vector.tensor_scalar_add(q2[:], pm2[:], fcb_t[c])
                p1_t.append(p1)
                q2_t.append(q2)

            for c in range(OC):
                chunk_list = _stripe_chunks(c)
                pieces = _chunk_pieces(chunk_list)
                coff = 0
                for ci, csz in enumerate(chunk_list):
                    ot = opool.tile([128, BIGCHUNK], f32, name="ot")
                    for (i, s0, s1, d0) in pieces[ci]:
                        nc.vector.tensor_scalar_add(
                            ot[:, d0 : d0 + (s1 - s0)],
                            q2_t[c][:, s0:s1],
                            p1_t[c][:, i : i + 1],
                        )
                    # separate ACT output tile: the out-DMA must depend on
                    # exactly one semaphore (ACT), not DVE+ACT via in-place
                    ot2 = opool.tile([128, BIGCHUNK], f32, name="ot2")
                    nc.scalar.activation(ot2[:, :csz], ot[:, :csz], Tanh, bias=zbias)
                    nc.sync.dma_start(
                        out_d[c, :, coff : coff + csz], ot2[:, :csz]
                    )
                    coff += csz
    nc.compile()
    return nc


def _get_nc():
    if "nc" not in _NC_CACHE:
        _NC_CACHE["nc"] = _build_nc()
    return _NC_CACHE["nc"]


def _make_in_maps(hidden_state, fc_w, fc_b):
    import ml_dtypes

    in_maps = []
    for k in range(8):
        b, h0 = k // 2, 384 * (k % 2)
        inp16 = np.empty((H, IC16), dtype=ml_dtypes.bfloat16)
        inp16[:, :S] = hidden_state[b].T.astype(ml_dtypes.bfloat16)
        inp16[:, W1C : W1C + 384] = fc_w[h0 : h0 + 384, :H].T.astype(
            ml_dtypes.bfloat16
        )
        inp16[:, W2C : W2C + 384] = fc_w[h0 : h0 + 384, H:].T.astype(
            ml_dtypes.bfloat16
        )
        aux = np.zeros((H, 2), dtype=np.float32)
        aux[: 128 * OC, 0] = fc_b[h0 : h0 + 384]
        in_maps.append(dict(inp16=inp16, aux=aux))
    return in_maps


def kernel(hidden_state, fc_w, fc_b, _trace=False, **_trace_kwargs):
    from concourse.bass_utils import run_bass_kernel_spmd

    hidden_state = np.asarray(hidden_state, dtype=np.float32)
    fc_w = np.asarray(fc_w, dtype=np.float32)
    fc_b = np.asarray(fc_b, dtype=np.float32)

    in_maps = _make_in_maps(hidden_state, fc_w, fc_b)
    nc = _get_nc()
    res = run_bass_kernel_spmd(
        nc, in_maps, core_ids=list(range(8)), trace=_trace, **_trace_kwargs
    )
    LAST["res"] = res

    full = np.empty((B, H, P), dtype=np.float32)
    for k in range(8):
        b, h0 = k // 2, 384 * (k % 2)
        full[b, h0 : h0 + 384] = res.results[k]["out"].reshape(384, P)
    return np.ascontiguousarray(full.transpose(0, 2, 1))


# revision 18
# speedup vs baseline: 1.3141x; 1.0183x over previous
"""Trainium2 Bass kernel for ConcatHandshaking.

out[b, p, :] = tanh(hidden[b, i_p] @ W1.T + hidden[b, j_p] @ W2.T + fc_b)
for the S*(S+1)/2 upper-triangular pairs (i_p, j_p), i-major order.

Device layout: output features (H=768) on SBUF partitions, pair index on the
free dim.  Then the pair-add is `q2T[:, j] + p1T[:, i]` where the second term
is a per-partition scalar -> one DVE tensor_scalar_add per triu segment,
fused bias, one big ACT tanh per output chunk, large contiguous DMA writes.

Sharding (8 cores): core k handles batch b = k//2 and output-feature rows
[384*(k%2), 384*(k%2)+384) -> 3 stripes of [128 features, 32896 pairs] each.
Per-core DRAM output is (3, 128, 32896); host reassembles + transposes.

Matmul operands ship as one bf16 tensor (PE 4x faster than f32; rel err
~1e-3 after f32 PSUM accumulation); fcb/zeros ship in a tiny f32 tensor.
The first stripe uses small leading chunks so the first output DMA starts
~12us in instead of waiting on a full 8224-wide chunk.
"""

import sys

import numpy as np

for _p in ("/opt/trn_rl_repo",):
    if _p not in sys.path:
        sys.path.insert(0, _p)

B, S, H = 4, 256, 768
P = S * (S + 1) // 2  # 32896
KT = H // 128  # 6 k-tiles
OC = 3  # o-chunks (of 128) per core
# bf16 packed matmul input columns: [ ht (S) | w1t (384) | w2t (384) ]
W1C = S
W2C = S + 128 * OC
IC16 = S + 2 * 128 * OC  # 1024
BIGCHUNK = 8224
SMALL = 2056

_NC_CACHE = {}
LAST = {}


def _stripe_chunks(c):
    if c == 0:
        # small leading chunks: first output DMA launches early
        return [1028, 1028, 2056, 2056, 2056, 8224, 8224, 8224]
    return [BIGCHUNK] * 4


def _chunk_pieces(chunk_list):
    """Split triu segments along chunk boundaries.

    Returns per-chunk lists of (i, src0, src1, dst0):
    chunk[:, dst0:dst0+(src1-src0)] = q2T[:, src0:src1] + p1T[:, i].
    """
    bounds = [0]
    for sz in chunk_list:
        bounds.append(bounds[-1] + sz)
    assert bounds[-1] == P
    pieces = [[] for _ in chunk_list]
    off = 0
    for i in range(S):
        seg0, seg1 = off, off + (S - i)
        off = seg1
        for ci, (c0, c1) in enumerate(zip(bounds[:-1], bounds[1:])):
            s = max(seg0, c0)
            e = min(seg1, c1)
            if e > s:
                src0 = i + (s - seg0)  # free index in q2T is j itself
                pieces[ci].append((i, src0, src0 + (e - s), s - c0))
    return pieces


def _build_nc(loop_k=None):
    import contextlib

    import concourse.bacc as bacc
    import concourse.mybir as mybir
    import concourse.tile as tile

    f32 = mybir.dt.float32
    bf16 = mybir.dt.bfloat16
    # Bacc (not raw Bass): its compile() runs generate_event_semaphores,
    # which splits multi-sem waits to satisfy TRN2's 1-wait-per-instruction.
    nc = bacc.Bacc()

    inp16_d = nc.declare_dram_parameter("inp16", [H, IC16], bf16, isOutput=False)
    # f32 side data: col 0 = fcb (rows 0:384), col 1 = zeros
    aux_d = nc.declare_dram_parameter("aux", [H, 2], f32, isOutput=False)
    out_d = nc.declare_dram_parameter("out", [OC, 128, P], f32, isOutput=True)

    Tanh = mybir.ActivationFunctionType.Tanh

    with tile.TileContext(nc) as tc:
        with (
            tc.tile_pool(name="const", bufs=1) as cpool,
            tc.tile_pool(name="mm", bufs=2, space="PSUM") as mpool,
            tc.tile_pool(name="outp", bufs=2) as opool,
            tc.For_i(0, loop_k, 1) if loop_k else contextlib.nullcontext(),
        ):
            # one DMA per k-tile so matmul kk can start as soon as its
            # k-tile lands (pipelines the load under the matmul chain)
            inp_b = cpool.tile([128, KT * IC16], bf16, name="inp_b")
            for kk in range(KT):
                nc.sync.dma_start(
                    inp_b[:, kk * IC16 : (kk + 1) * IC16],
                    inp16_d[kk * 128 : (kk + 1) * 128, :],
                )
            aux_b = cpool.tile([128, KT * 2], f32, name="aux_b")
            nc.sync.dma_start(
                aux_b[:].rearrange("p (t c) -> p t c", t=KT),
                aux_d.rearrange("(t p) c -> p t c", p=128),
            )
            # block kk occupies cols [kk*IC16, (kk+1)*IC16)
            ht_t = [inp_b[:, kk * IC16 : kk * IC16 + S] for kk in range(KT)]
            fcb_t = [aux_b[:, c * 2 : c * 2 + 1] for c in range(OC)]

            p1_t, q2_t = [], []
            for c in range(OC):
                pm1 = mpool.tile([128, S], f32, name="pm1")
                pm2 = mpool.tile([128, S], f32, name="pm2")
                for kk in range(KT):
                    nc.tensor.matmul(
                        pm1[:],
                        inp_b[
                            :, kk * IC16 + W1C + c * 128 : kk * IC16 + W1C + (c + 1) * 128
                        ],
                        ht_t[kk],
                        start=(kk == 0),
                        stop=(kk == KT - 1),
                    )
                for kk in range(KT):
                    nc.tensor.matmul(
                        pm2[:],
                        inp_b[
                            :, kk * IC16 + W2C + c * 128 : kk * IC16 + W2C + (c + 1) * 128
                        ],
                        ht_t[kk],
                        start=(kk == 0),
                        stop=(kk == KT - 1),
                    )
                p1 = cpool.tile([128, S], f32, name=f"p1_{c}")
                q2 = cpool.tile([128, S], f32, name=f"q2_{c}")
                nc.vector.tensor_copy(p1[:], pm1[:])
                nc.vector.tensor_scalar_add(q2[:], pm2[:], fcb_t[c])
                p1_t.append(p1)
                q2_t.append(q2)

            for c in range(OC):
                chunk_list = _stripe_chunks(c)
                pieces = _chunk_pieces(chunk_list)
                coff = 0
                for ci, csz in enumerate(chunk_list):
                    ot = opool.tile([128, BIGCHUNK], f32, name="ot")
                    for (i, s0, s1, d0) in pieces[ci]:
                        nc.vector.tensor_scalar_add(
                            ot[:, d0 : d0 + (s1 - s0)],
                            q2_t[c][:, s0:s1],
                            p1_t[c][:, i : i + 1],
                        )
                    # separate ACT output tile: the out-DMA must depend on
                    # exactly one semaphore (ACT), not DVE+ACT via in-place
                    ot2 = opool.tile([128, BIGCHUNK], f32, name="ot2")
                    nc.scalar.activation(ot2[:, :csz], ot[:, :csz], Tanh)
                    nc.sync.dma_start(
                        out_d[c, :, coff : coff + csz], ot2[:, :csz]
                    )
                    coff += csz
    nc.compile()
    return nc


def _get_nc():
    if "nc" not in _NC_CACHE:
        _NC_CACHE["nc"] = _build_nc()
    return _NC_CACHE["nc"]


def _make_in_maps(hidden_state, fc_w, fc_b):
    import ml_dtypes

    in_maps = []
    for k in range(8):
        b, h0 = k // 2, 384 * (k % 2)
        inp16 = np.empty((H, IC16), dtype=ml_dtypes.bfloat16)
        inp16[:, :S] = hidden_state[b].T.astype(ml_dtypes.bfloat16)
        inp16[:, W1C : W1C + 384] = fc_w[h0 : h0 + 384, :H].T.astype(
            ml_dtypes.bfloat16
        )
        inp16[:, W2C : W2C + 384] = fc_w[h0 : h0 + 384, H:].T.astype(
            ml_dtypes.bfloat16
        )
        aux = np.zeros((H, 2), dtype=np.float32)
        aux[: 128 * OC, 0] = fc_b[h0 : h0 + 384]
        in_maps.append(dict(inp16=inp16, aux=aux))
    return in_maps


def kernel(hidden_state, fc_w, fc_b, _trace=False, **_trace_kwargs):
    from concourse.bass_utils import run_bass_kernel_spmd

    hidden_state = np.asarray(hidden_state, dtype=np.float32)
    fc_w = np.asarray(fc_w, dtype=np.float32)
    fc_b = np.asarray(fc_b, dtype=np.float32)

    in_maps = _make_in_maps(hidden_state, fc_w, fc_b)
    nc = _get_nc()
    res = run_bass_kernel_spmd(
        nc, in_maps, core_ids=list(range(8)), trace=_trace, **_trace_kwargs
    )
    LAST["res"] = res

    full = np.empty((B, H, P), dtype=np.float32)
    for k in range(8):
        b, h0 = k // 2, 384 * (k % 2)
        full[b, h0 : h0 + 384] = res.results[k]["out"].reshape(384, P)
    return np.ascontiguousarray(full.transpose(0, 2, 1))


# revision 25
# speedup vs baseline: 1.3195x; 1.0041x over previous
"""Trainium2 Bass kernel for ConcatHandshaking.

out[b, p, :] = tanh(hidden[b, i_p] @ W1.T + hidden[b, j_p] @ W2.T + fc_b)
for the S*(S+1)/2 upper-triangular pairs (i_p, j_p), i-major order.

Device layout: output features (H=768) on SBUF partitions, pair index on the
free dim.  Then the pair-add is `q2T[:, j] + p1T[:, i]` where the second term
is a per-partition scalar -> one DVE tensor_scalar_add per triu segment,
fused bias, one big ACT tanh per output chunk, large contiguous DMA writes.

Sharding (8 cores): core k handles batch b = k//2 and output-feature rows
[384*(k%2), 384*(k%2)+384) -> 3 stripes of [128 features, 32896 pairs] each.
Per-core DRAM output is (3, 128, 32896); host reassembles + transposes.

Matmul operands ship as one bf16 tensor (PE 4x faster than f32; rel err
~1e-3 after f32 PSUM accumulation); fcb/zeros ship in a tiny f32 tensor.
The first stripe uses small leading chunks so the first output DMA starts
~12us in instead of waiting on a full 8224-wide chunk.
"""

import sys

import numpy as np

for _p in ("/opt/trn_rl_repo",):
    if _p not in sys.path:
        sys.path.insert(0, _p)

B, S, H = 4, 256, 768
P = S * (S + 1) // 2  # 32896
KT = H // 128  # 6 k-tiles
OC = 3  # o-chunks (of 128) per core
# bf16 packed matmul input columns: [ ht (S) | w1t (384) | w2t (384) ]
W1C = S
W2C = S + 128 * OC
IC16 = S + 2 * 128 * OC  # 1024
BIGCHUNK = 8224
SMALL = 2056

_NC_CACHE = {}
LAST = {}


def _stripe_chunks(c):
    if c == 0:
        # geometric-ish leading chunks: first output DMA launches early and
        # the stream never stalls waiting on one big chunk's DVE+ACT latency
        return [1028, 1028, 2056, 2056, 2056] + [4112] * 6
    return [BIGCHUNK] * 4


def _chunk_pieces(chunk_list):
    """Split triu segments along chunk boundaries.

    Returns per-chunk lists of (i, src0, src1, dst0):
    chunk[:, dst0:dst0+(src1-src0)] = q2T[:, src0:src1] + p1T[:, i].
    """
    bounds = [0]
    for sz in chunk_list:
        bounds.append(bounds[-1] + sz)
    assert bounds[-1] == P
    pieces = [[] for _ in chunk_list]
    off = 0
    for i in range(S):
        seg0, seg1 = off, off + (S - i)
        off = seg1
        for ci, (c0, c1) in enumerate(zip(bounds[:-1], bounds[1:])):
            s = max(seg0, c0)
            e = min(seg1, c1)
            if e > s:
                src0 = i + (s - seg0)  # free index in q2T is j itself
                pieces[ci].append((i, src0, src0 + (e - s), s - c0))
    return pieces


def _build_nc(loop_k=None):
    import contextlib

    import concourse.bacc as bacc
    import concourse.mybir as mybir
    import concourse.tile as tile

    f32 = mybir.dt.float32
    bf16 = mybir.dt.bfloat16
    # Bacc (not raw Bass): its compile() runs generate_event_semaphores,
    # which splits multi-sem waits to satisfy TRN2's 1-wait-per-instruction.
    nc = bacc.Bacc()

    inp16_d = nc.declare_dram_parameter("inp16", [H, IC16], bf16, isOutput=False)
    # f32 side data: col 0 = fcb (rows 0:384), col 1 = zeros
    aux_d = nc.declare_dram_parameter("aux", [H, 2], f32, isOutput=False)
    out_d = nc.declare_dram_parameter("out", [OC, 128, P], f32, isOutput=True)

    Tanh = mybir.ActivationFunctionType.Tanh

    with tile.TileContext(nc) as tc:
        with (
            tc.tile_pool(name="const", bufs=1) as cpool,
            tc.tile_pool(name="mm", bufs=2, space="PSUM") as mpool,
            tc.tile_pool(name="outp", bufs=2) as opool,
            tc.tile_pool(name="outp2", bufs=3) as opool2,
            tc.For_i(0, loop_k, 1) if loop_k else contextlib.nullcontext(),
        ):
            # one DMA per k-tile so matmul kk can start as soon as its
            # k-tile lands (pipelines the load under the matmul chain)
            inp_b = cpool.tile([128, KT * IC16], bf16, name="inp_b")
            for kk in range(KT):
                nc.sync.dma_start(
                    inp_b[:, kk * IC16 : (kk + 1) * IC16],
                    inp16_d[kk * 128 : (kk + 1) * 128, :],
                )
            aux_b = cpool.tile([128, KT * 2], f32, name="aux_b")
            nc.sync.dma_start(
                aux_b[:].rearrange("p (t c) -> p t c", t=KT),
                aux_d.rearrange("(t p) c -> p t c", p=128),
            )
            # block kk occupies cols [kk*IC16, (kk+1)*IC16)
            ht_t = [inp_b[:, kk * IC16 : kk * IC16 + S] for kk in range(KT)]
            fcb_t = [aux_b[:, c * 2 : c * 2 + 1] for c in range(OC)]

            for c in range(OC):
                pm1 = mpool.tile([128, S], f32, name="pm1")
                pm2 = mpool.tile([128, S], f32, name="pm2")
                for kk in range(KT):
                    nc.tensor.matmul(
                        pm1[:],
                        inp_b[
                            :, kk * IC16 + W1C + c * 128 : kk * IC16 + W1C + (c + 1) * 128
                        ],
                        ht_t[kk],
                        start=(kk == 0),
                        stop=(kk == KT - 1),
                    )
                for kk in range(KT):
                    nc.tensor.matmul(
                        pm2[:],
                        inp_b[
                            :, kk * IC16 + W2C + c * 128 : kk * IC16 + W2C + (c + 1) * 128
                        ],
                        ht_t[kk],
                        start=(kk == 0),
                        stop=(kk == KT - 1),
                    )
                p1 = cpool.tile([128, S], f32, name=f"p1_{c}")
                q2 = cpool.tile([128, S], f32, name=f"q2_{c}")
                nc.vector.tensor_copy(p1[:], pm1[:])
                nc.vector.tensor_scalar_add(q2[:], pm2[:], fcb_t[c])

                chunk_list = _stripe_chunks(c)
                pieces = _chunk_pieces(chunk_list)
                coff = 0
                for ci, csz in enumerate(chunk_list):
                    ot = opool.tile([128, BIGCHUNK], f32, name="ot")
                    for (i, s0, s1, d0) in pieces[ci]:
                        nc.vector.tensor_scalar_add(
                            ot[:, d0 : d0 + (s1 - s0)],
                            q2[:, s0:s1],
                            p1[:, i : i + 1],
                        )
                    # separate ACT output tile: the out-DMA must depend on
                    # exactly one semaphore (ACT), not DVE+ACT via in-place
                    ot2 = opool2.tile([128, BIGCHUNK], f32, name="ot2")
                    nc.scalar.activation(ot2[:, :csz], ot[:, :csz], Tanh)
                    nc.sync.dma_start(
                        out_d[c, :, coff : coff + csz], ot2[:, :csz]
                    )
                    coff += csz
    nc.compile()
    return nc


def _get_nc():
    if "nc" not in _NC_CACHE:
        _NC_CACHE["nc"] = _build_nc()
    return _NC_CACHE["nc"]


def _make_in_maps(hidden_state, fc_w, fc_b):
    import ml_dtypes

    in_maps = []
    for k in range(8):
        b, h0 = k // 2, 384 * (k % 2)
        inp16 = np.empty((H, IC16), dtype=ml_dtypes.bfloat16)
        inp16[:, :S] = hidden_state[b].T.astype(ml_dtypes.bfloat16)
        inp16[:, W1C : W1C + 384] = fc_w[h0 : h0 + 384, :H].T.astype(
            ml_dtypes.bfloat16
        )
        inp16[:, W2C : W2C + 384] = fc_w[h0 : h0 + 384, H:].T.astype(
            ml_dtypes.bfloat16
        )
        aux = np.zeros((H, 2), dtype=np.float32)
        aux[: 128 * OC, 0] = fc_b[h0 : h0 + 384]
        in_maps.append(dict(inp16=inp16, aux=aux))
    return in_maps


def kernel(hidden_state, fc_w, fc_b, _trace=False, **_trace_kwargs):
    from concourse.bass_utils import run_bass_kernel_spmd

    hidden_state = np.asarray(hidden_state, dtype=np.float32)
    fc_w = np.asarray(fc_w, dtype=np.float32)
    fc_b = np.asarray(fc_b, dtype=np.float32)

    in_maps = _make_in_maps(hidden_state, fc_w, fc_b)
    nc = _get_nc()
    res = run_bass_kernel_spmd(
        nc, in_maps, core_ids=list(range(8)), trace=_trace, **_trace_kwargs
    )
    LAST["res"] = res

    full = np.empty((B, H, P), dtype=np.float32)
    for k in range(8):
        b, h0 = k // 2, 384 * (k % 2)
        full[b, h0 : h0 + 384] = res.results[k]["out"].reshape(384, P)
    return np.ascontiguousarray(full.transpose(0, 2, 1))
